# revision 4
# baseline (speedup 1.0000x reference)
"""Trainium2 Bass kernel for nn_CNNModel (ragged resize -> conv1d -> pools -> MLP).

Self-contained: hardcodes shapes B=64, N=256, L=1024, TARGET=100, 8 cores.
Pure data parallel over batch (2048 seqs/core).

v2 path: host sorts seqs by length so 16-seq gather groups share W=16 windows
on pair-encoded tokens u[l] = t[l] + 8*t[l+1]; 4 macro-tiles of 512 seqs.
Device: SWDGE int32->bf16 token DMA -> u-encode (DVE stt) -> gpsimd
indirect_copy window gathers (d=8 int32 view) -> DMA'd one-hot extract +
tree-reduce -> is_ge decode -> phi features + frac blend (all DVE) -> 4-chunk
block-Toeplitz conv matmuls (bias via ones channel) -> relu-copy + tree
maxpool -> block-diag dense1 + global max -> block-diag dense2 -> sigmoid
classifier -> per-macro PSUM S-matmul batch attribution; 3-phase software
pipeline (head / tail_conv / tail_rest) across macros.

Fallback (v1, exact 16x-redundant gather) if any gather group's window
offsets exceed [0, 15] (host-verified per run).
"""

from contextlib import ExitStack

import numpy as np

B, N, L = 64, 256, 1024
TARGET = 100
KW = 8
N_CORES = 8
SEQ_PER_CORE = 2048
MS = 4
MACROS = 4
W = 16
CHUNKS = [(0, 24), (24, 24), (48, 24), (72, 16)]
POFF = [0, 3, 6, 9]
NW = MS * TARGET

# v1 fallback constants
NPOOL_V1 = 11
TILES_V1 = SEQ_PER_CORE // 128
CHUNKS_V1 = [(0, 24), (24, 24), (48, 24), (72, 16)]

_CACHE = {}

# set by test.py to capture an NTFF profile; harness leaves these alone
TRACE = False
TRACE_DIR = None
LAST_EXEC_NS = None
LAST_TRACE_PATH = None


def _bf16():
    import ml_dtypes
    return ml_dtypes.bfloat16


# ----------------------------------------------------------------------------
# host-side
# ----------------------------------------------------------------------------

def _resize_tables(lengths_flat):
    lf = lengths_flat.astype(np.float64)[:, None]
    i = np.arange(TARGET, dtype=np.float64)[None, :]
    src = (i + 0.5) * lf / TARGET - 0.5
    f = np.floor(src)
    lo = np.clip(f, 0, lf - 2).astype(np.int64)
    fr = np.clip(src - lo, 0.0, 1.0)
    return lo, fr.astype(np.float32)


def _fold_weights(embed_w, conv_w, conv_b):
    Phi = np.array([[1, 1, 0, 0], [1, 2, 0, 0], [1, 3, 1, 0], [1, 4, 2, 1]],
                   dtype=np.float64)
    E = embed_w[1:5].astype(np.float64)
    M = np.linalg.solve(Phi, E)
    wf = np.einsum("fc,kco->kfo", M[1:], conv_w.astype(np.float64))
    bias = conv_b.astype(np.float64) + np.einsum("c,kco->o", M[0],
                                                 conv_w.astype(np.float64))
    return wf.astype(np.float32), bias.astype(np.float32)


def build_consts(embed_w, conv_w, conv_b, w1, b1, w2, b2, wc, bc):
    bf16 = _bf16()
    wf, bconv = _fold_weights(embed_w, conv_w, conv_b)
    Wcs = []
    for (st, opc) in CHUNKS:
        span = opc + KW - 1
        Wm = np.zeros((4 * span, opc * 12), np.float32)
        for dlt in range(span):
            for o in range(opc):
                k = dlt - o
                if 0 <= k < KW:
                    for f in range(3):
                        Wm[dlt * 4 + f, o * 12:(o + 1) * 12] = wf[k, f, :]
                if k == 0:
                    Wm[dlt * 4 + 3, o * 12:(o + 1) * 12] = bconv
        Wcs.append(Wm.astype(bf16))
    R1A = np.zeros((72, 192), np.float32)
    R1B = np.zeros((60, 160), np.float32)
    for p in range(6):
        R1A[12 * p:12 * p + 12, 32 * p:32 * p + 32] = w1
    for p in range(5):
        R1B[12 * p:12 * p + 12, 32 * p:32 * p + 32] = w1
    W2blk = np.zeros((128, MS * 64), np.float32)
    for s in range(MS):
        W2blk[s * 32:(s + 1) * 32, s * 64:(s + 1) * 64] = w2
    wcd = (wc[:, 1] - wc[:, 0]).astype(np.float32)
    bcd = float(bc[1] - bc[0])
    consts = {
        "r1a": R1A.astype(bf16), "r1b": R1B.astype(bf16),
        "w2blk": W2blk.astype(bf16),
        "b1rep": np.tile(b1.astype(np.float32), MS)[None, :]
                 .repeat(128, 0).astype(bf16),
        "b2rep": np.tile(b2.astype(np.float32), MS)[None, :]
                 .repeat(128, 0).astype(bf16),
        "wcdrep": np.tile(wcd, MS)[None, :].repeat(128, 0).astype(bf16),
        "iota16": np.arange(16, dtype=np.float32)[None, :]
                  .repeat(128, 0).astype(bf16),
        "identb": np.eye(128, dtype=np.float32).astype(bf16),
        "biasv": np.tile(np.array([-2.0, -3.0, bcd, -bcd, 0.0, 0.0, 0.0, 0.0],
                                  np.float32)[None, :], (128, 1)),
    }
    for i, W in enumerate(Wcs):
        consts[f"w_c{i}"] = W
    return consts, bcd


def plan_core(tokens_core, lengths_core):
    """-> (dict of per-macro arrays, max_o). Window bases are even (int32
    gather units); o = lo - base must be in [0, 15]."""
    bf16 = _bf16()
    lo, fr = _resize_tables(lengths_core)
    perm = np.argsort(lengths_core, kind="stable")
    out = {}
    max_o = 0
    jj = np.arange(NW)
    ss, ii = jj // TARGET, jj % TARGET
    for m in range(MACROS):
        sm = np.zeros((128, MS), np.int64)
        for g in range(8):
            for s in range(MS):
                c = m * 32 + g * 4 + s
                sm[16 * g:16 * g + 16, s] = perm[16 * c:16 * c + 16]
        lo_m = lo[sm]                                  # [128, MS, 100]
        fr_m = fr[sm]
        lo_g = lo_m.reshape(8, 16, MS, TARGET)
        lomin = lo_g.min(axis=1)                       # [8, MS, 100]
        base = np.minimum(lomin - (lomin % 2), 1008)
        o = lo_g - base[:, None]                       # [8, 16, MS, 100]
        max_o = max(max_o, int(o.max()))
        idx = np.zeros((128, 28), np.uint16)
        j25 = np.arange(25)
        for g in range(8):
            for k in range(16):
                j = j25 * 16 + k
                idx[16 * g + k, :25] = \
                    (ss[j] * L + base[g, ss[j], ii[j]]) // 2
        out[f"tok{m}"] = np.ascontiguousarray(
            tokens_core[sm].reshape(128, MS * L))
        out[f"idx{m}"] = idx
        ot = np.ascontiguousarray(
            o.transpose(0, 1, 2, 3).reshape(128, NW))
        out[f"oh{m}"] = (ot[:, :, None] ==
                         np.arange(W)[None, None, :]).astype(bf16)
        out[f"fr{m}"] = np.ascontiguousarray(
            fr_m.reshape(128, NW)).astype(bf16)
        S = np.zeros((128, MS, 8), np.float32)
        bidx = sm // N
        for s in range(MS):
            S[np.arange(128), s, bidx[:, s]] = 1.0
        out[f"sb{m}"] = S.astype(bf16)
    return out, max_o


# ----------------------------------------------------------------------------
# device program
# ----------------------------------------------------------------------------

def build_program(bcd):
    import concourse.bass as bass  # noqa: F401
    import concourse.tile as tile
    from concourse import bacc, mybir

    dt = mybir.dt
    Alu = mybir.AluOpType
    Act = mybir.ActivationFunctionType
    Ax = mybir.AxisListType

    nc = bacc.Bacc("TRN2", target_bir_lowering=False, debug=False)

    def din(name, shape, dtype):
        return nc.dram_tensor(name, shape, dtype, kind="ExternalInput").ap()

    tok_d = [din(f"tok{m}", [128, MS * L], dt.int32) for m in range(MACROS)]
    idx_d = [din(f"idx{m}", [128, 28], dt.uint16) for m in range(MACROS)]
    oh_d = [din(f"oh{m}", [128, NW, W], dt.bfloat16) for m in range(MACROS)]
    fr_d = [din(f"fr{m}", [128, NW], dt.bfloat16) for m in range(MACROS)]
    sb_d = [din(f"sb{m}", [128, MS, 8], dt.bfloat16) for m in range(MACROS)]
    wc_d = [din(f"w_c{i}", [4 * (opc + 7), opc * 12], dt.bfloat16)
            for i, (_, opc) in enumerate(CHUNKS)]
    r1a_d = din("r1a", [72, 192], dt.bfloat16)
    r1b_d = din("r1b", [60, 160], dt.bfloat16)
    b1rep_d = din("b1rep", [128, MS * 32], dt.bfloat16)
    w2blk_d = din("w2blk", [128, MS * 64], dt.bfloat16)
    b2rep_d = din("b2rep", [128, MS * 64], dt.bfloat16)
    wcdrep_d = din("wcdrep", [128, MS * 64], dt.bfloat16)
    identb_d = din("identb", [128, 128], dt.bfloat16)
    biasv_d = din("biasv", [128, 8], dt.float32)
    out_d = nc.dram_tensor("out", [B // N_CORES, 2], dt.float32,
                           kind="ExternalOutput").ap()

    with tile.TileContext(nc) as tc, ExitStack() as ctx:
        cpool = ctx.enter_context(tc.tile_pool(name="consts", bufs=1))
        iop = ctx.enter_context(tc.tile_pool(name="io", bufs=4))
        auxp = ctx.enter_context(tc.tile_pool(name="aux", bufs=4))
        ohp = ctx.enter_context(tc.tile_pool(name="ohp", bufs=2))
        lhp = ctx.enter_context(tc.tile_pool(name="lhs", bufs=2))
        wp = ctx.enter_context(tc.tile_pool(name="work", bufs=2))
        sp = ctx.enter_context(tc.tile_pool(name="single", bufs=1))
        ps_tp = ctx.enter_context(tc.tile_pool(name="ps_tp", bufs=3,
                                               space="PSUM"))
        ps_mm = ctx.enter_context(tc.tile_pool(name="ps_mm", bufs=2,
                                               space="PSUM"))
        ps_h1 = ctx.enter_context(tc.tile_pool(name="ps_h1", bufs=2,
                                               space="PSUM"))

        def cload(ap_d, shape, dtype, tag):
            t = cpool.tile(shape, dtype, tag=tag)
            nc.sync.dma_start(t[:], ap_d[:])
            return t

        Wc = [cload(wc_d[i], [4 * (opc + 7), opc * 12], dt.bfloat16,
                    f"cW{i}") for i, (_, opc) in enumerate(CHUNKS)]
        R1A = cload(r1a_d, [72, 192], dt.bfloat16, "cR1A")
        R1B = cload(r1b_d, [60, 160], dt.bfloat16, "cR1B")
        B1REP = cload(b1rep_d, [128, MS * 32], dt.bfloat16, "cB1")
        W2BLK = cload(w2blk_d, [128, MS * 64], dt.bfloat16, "cW2B")
        B2REP = cload(b2rep_d, [128, MS * 64], dt.bfloat16, "cB2")
        WCDREP = cload(wcdrep_d, [128, MS * 64], dt.bfloat16, "cWCD")
        IDENT = cload(identb_d, [128, 128], dt.bfloat16, "cID")
        BIASV = cload(biasv_d, [128, 8], dt.float32, "cBV")
        ONE400 = cpool.tile([128, NW], dt.bfloat16, tag="cONE")
        nc.vector.memset(ONE400[:], 1.0)

        acc_sb = cpool.tile([2, 8], dt.float32, tag="accsb")
        nc.vector.memset(acc_sb[:], 0.0)

        def load_dmas(m):
            tokb = iop.tile([128, MS * L], dt.bfloat16, tag="tokb")
            nc.gpsimd.dma_start(tokb[:], tok_d[m][:])
            idxt = auxp.tile([128, 28], dt.uint16, tag="idxt")
            nc.scalar.dma_start(idxt[:], idx_d[m][:])
            frt = auxp.tile([128, NW], dt.bfloat16, tag="frt")
            nc.scalar.dma_start(frt[:], fr_d[m][:])
            sbt = auxp.tile([128, MS, 8], dt.bfloat16, tag="sbt")
            nc.scalar.dma_start(sbt[:], sb_d[m][:])
            oht = ohp.tile([128, NW, W], dt.bfloat16, tag="oht")
            nc.sync.dma_start(oht[:], oh_d[m][:])
            return dict(tokb=tokb, idxt=idxt, oht=oht, frt=frt, sbt=sbt)

        def encode(st):
            tokb = st["tokb"]
            u = iop.tile([128, MS * L], dt.bfloat16, tag="u")
            nv = MS * L - 1
            # u[l] = 8*t[l+1] + t[l]  (one DVE scalar_tensor_tensor)
            nc.vector.memset(u[:, nv:nv + 1], 9.0)
            nc.vector.scalar_tensor_tensor(out=u[:, 0:nv],
                                           in0=tokb[:, 1:MS * L], scalar=8.0,
                                           in1=tokb[:, 0:nv], op0=Alu.mult,
                                           op1=Alu.add)
            st["u"] = u

        def gather(st):
            win = wp.tile([128, 448, W], dt.bfloat16, tag="win")
            u32 = st["u"][:].bitcast(dt.int32).rearrange(
                "p (n d) -> p n d", d=W // 2)
            w32 = win[:].bitcast(dt.int32)
            for j in range(7):
                nc.gpsimd.indirect_copy(
                    w32[:, 64 * j:64 * j + 64, :], u32,
                    st["idxt"][:, 4 * j:4 * j + 4],
                    i_know_ap_gather_is_preferred=True)
            st["win"] = win
            return st

        def compute_head(st, m):
            win, oh, frt = st["win"], st["oht"], st["frt"]
            prod = sp.tile([128, NW, W], dt.bfloat16, tag="prod")
            nc.vector.tensor_tensor(out=prod[:], in0=win[:, 0:NW, :],
                                    in1=oh[:], op=Alu.mult)
            t1 = sp.tile([128, NW, 8], dt.bfloat16, tag="t1")
            nc.vector.tensor_tensor(out=t1[:], in0=prod[:, :, 0:8],
                                    in1=prod[:, :, 8:16], op=Alu.add)
            t2 = sp.tile([128, NW, 4], dt.bfloat16, tag="t2")
            nc.vector.tensor_tensor(out=t2[:], in0=t1[:, :, 0:4],
                                    in1=t1[:, :, 4:8], op=Alu.add)
            t3 = sp.tile([128, NW, 2], dt.bfloat16, tag="t3")
            nc.vector.tensor_tensor(out=t3[:], in0=t2[:, :, 0:2],
                                    in1=t2[:, :, 2:4], op=Alu.add)
            usel = sp.tile([128, NW], dt.bfloat16, tag="usel")
            nc.vector.tensor_tensor(
                out=usel[:].rearrange("p (i a) -> p i a", a=1),
                in0=t3[:, :, 0:1], in1=t3[:, :, 1:2], op=Alu.add)
            # --- decode u = t_lo + 8*t_hi (phi tiles are x-major [p, NW, 3]) ---
            phiL = sp.tile([128, NW, 3], dt.bfloat16, tag="phiL")
            phiH = sp.tile([128, NW, 3], dt.bfloat16, tag="phiH")
            tlv = phiL[:, :, 0:1].rearrange("p a b -> p (a b)")
            thv = phiH[:, :, 0:1].rearrange("p a b -> p (a b)")
            dA = sp.tile([128, NW], dt.bfloat16, tag="dA")
            nc.vector.scalar_tensor_tensor(out=dA[:], in0=usel[:],
                                           scalar=32.0, in1=ONE400[:],
                                           op0=Alu.is_ge, op1=Alu.add)
            dB = sp.tile([128, NW], dt.bfloat16, tag="dB")
            nc.vector.scalar_tensor_tensor(out=dB[:], in0=usel[:],
                                           scalar=24.0, in1=dA[:],
                                           op0=Alu.is_ge, op1=Alu.add)
            nc.vector.scalar_tensor_tensor(out=thv, in0=usel[:],
                                           scalar=16.0, in1=dB[:],
                                           op0=Alu.is_ge, op1=Alu.add)
            nc.vector.scalar_tensor_tensor(out=tlv, in0=thv, scalar=-8.0,
                                           in1=usel[:], op0=Alu.mult,
                                           op1=Alu.add)
            # --- phi features (DVE tensor_scalar: relu(x-c)) ---
            for buf, src in ((phiL, tlv), (phiH, thv)):
                nc.vector.tensor_scalar(
                    out=buf[:, :, 1:2].rearrange("p a b -> p (a b)"),
                    in0=src, scalar1=2.0, scalar2=0.0,
                    op0=Alu.subtract, op1=Alu.max)
                nc.vector.tensor_scalar(
                    out=buf[:, :, 2:3].rearrange("p a b -> p (a b)"),
                    in0=src, scalar1=3.0, scalar2=0.0,
                    op0=Alu.subtract, op1=Alu.max)
            D = sp.tile([128, NW, 3], dt.bfloat16, tag="D")
            nc.vector.tensor_tensor(out=D[:], in0=phiH[:],
                                    in1=phiL[:, :, 0:3], op=Alu.subtract)
            D2 = sp.tile([128, NW, 3], dt.bfloat16, tag="D2")
            nc.vector.tensor_tensor(
                out=D2[:], in0=D[:],
                in1=frt[:].rearrange("p (j a) -> p j a", a=1)
                    .to_broadcast([128, NW, 3]), op=Alu.mult)
            F = wp.tile([128, NW, 4], dt.bfloat16, tag="F")
            nc.vector.memset(F[:, :, 3:4], 1.0)
            nc.vector.tensor_tensor(out=F[:, :, 0:3], in0=phiL[:, :, 0:3],
                                    in1=D2[:], op=Alu.add)
            st["F"] = F

        def tail_conv(st, m):
            F = st["F"]
            # --- conv chunks -> relu-copy ---
            ybuf = wp.tile([128, MS, 11 * 96], dt.bfloat16, tag="ybuf")
            for s in range(MS):
                for ci, (st_c, opc) in enumerate(CHUNKS):
                    span = opc + 7
                    rows = 4 * span
                    tp = ps_tp.tile([128, 128], dt.bfloat16, tag="tp")
                    nc.tensor.transpose(
                        out=tp[0:rows, :],
                        in_=F[:, s * TARGET + st_c:s * TARGET + st_c + span,
                              :].rearrange("p x f -> p (x f)"),
                        identity=IDENT[:])
                    xtc = lhp.tile([124, 128], dt.bfloat16, tag="xtc")
                    nc.scalar.copy(out=xtc[0:rows, :], in_=tp[0:rows, :])
                    y = ps_mm.tile([128, 384], dt.float32, tag="ymm")
                    nc.tensor.matmul(out=y[:, 0:opc * 12],
                                     lhsT=xtc[0:rows, :],
                                     rhs=Wc[ci][:], start=True, stop=True)
                    nc.scalar.activation(
                        out=ybuf[:, s:s + 1, POFF[ci] * 96:
                                 (POFF[ci] + opc // 8) * 96]
                            .rearrange("p a b -> p (a b)"),
                        in_=y[:, 0:opc * 12], func=Act.Relu,
                        bias=BIASV[:, 4:5])
            st["ybuf"] = ybuf

        def tail_rest(st, m):
            ybuf, sbt = st["ybuf"], st["sbt"]
            # maxpool(8) tree over q
            yv = ybuf[:].rearrange("p s (pl q c) -> p (s pl) q c", q=8, c=12)
            p1 = sp.tile([128, 44, 4, 12], dt.bfloat16, tag="p1")
            nc.vector.tensor_tensor(out=p1[:], in0=yv[:, :, 0:4, :],
                                    in1=yv[:, :, 4:8, :], op=Alu.max)
            p2 = sp.tile([128, 44, 2, 12], dt.bfloat16, tag="p2")
            nc.vector.tensor_tensor(out=p2[:], in0=p1[:, :, 0:2, :],
                                    in1=p1[:, :, 2:4, :], op=Alu.max)
            hr = sp.tile([128, 44, 12], dt.bfloat16, tag="hr")
            nc.vector.tensor_tensor(
                out=hr[:],
                in0=p2[:, :, 0:1, :].rearrange("p a b c -> p (a b) c"),
                in1=p2[:, :, 1:2, :].rearrange("p a b c -> p (a b) c"),
                op=Alu.max)
            # --- dense1 (block-diag) + global max ---
            gar = sp.tile([128, 128], dt.bfloat16, tag="gar")
            for s in range(MS):
                hs = hr[:, s * 11:(s + 1) * 11, :]
                tpA = ps_tp.tile([128, 128], dt.bfloat16, tag="tp")
                nc.tensor.transpose(out=tpA[0:72, :],
                                    in_=hs[:, 0:6, :], identity=IDENT[:])
                htA = sp.tile([72, 128], dt.bfloat16, tag="htA")
                nc.scalar.copy(out=htA[:], in_=tpA[0:72, :])
                tpB = ps_tp.tile([128, 128], dt.bfloat16, tag="tp")
                nc.tensor.transpose(out=tpB[0:60, :],
                                    in_=hs[:, 6:11, :], identity=IDENT[:])
                htB = sp.tile([60, 128], dt.bfloat16, tag="htB")
                nc.scalar.copy(out=htB[:], in_=tpB[0:60, :])
                h1a = ps_h1.tile([128, 192], dt.float32, tag="h1")
                nc.tensor.matmul(out=h1a[:], lhsT=htA[:], rhs=R1A[:],
                                 start=True, stop=True)
                h1b = ps_h1.tile([128, 192], dt.float32, tag="h1")
                nc.tensor.matmul(out=h1b[:, 0:160], lhsT=htB[:], rhs=R1B[:],
                                 start=True, stop=True)
                ga6 = sp.tile([128, 32], dt.bfloat16, tag="ga6")
                nc.vector.tensor_reduce(
                    out=ga6[:], in_=h1a[:].rearrange("p (g o) -> p o g", g=6),
                    axis=Ax.X, op=Alu.max)
                gb5 = sp.tile([128, 32], dt.bfloat16, tag="gb5")
                nc.vector.tensor_reduce(
                    out=gb5[:],
                    in_=h1b[:, 0:160].rearrange("p (g o) -> p o g", g=5),
                    axis=Ax.X, op=Alu.max)
                nc.vector.tensor_tensor(out=gar[:, s * 32:(s + 1) * 32],
                                        in0=ga6[:], in1=gb5[:], op=Alu.max)
            # --- dense2 (block-diag over slots) + classifier ---
            garb = sp.tile([128, 128], dt.bfloat16, tag="garb")
            nc.vector.tensor_tensor(out=garb[:], in0=gar[:], in1=B1REP[:],
                                    op=Alu.add)
            garr = sp.tile([128, 128], dt.bfloat16, tag="garr")
            nc.scalar.activation(out=garr[:], in_=garb[:], func=Act.Relu,
                                 bias=BIASV[:, 4:5])
            tpg = ps_tp.tile([128, 128], dt.bfloat16, tag="tp")
            nc.tensor.transpose(out=tpg[:], in_=garr[:], identity=IDENT[:])
            gaT = sp.tile([128, 128], dt.bfloat16, tag="gaT")
            nc.scalar.copy(out=gaT[:], in_=tpg[:])
            r2 = ps_mm.tile([128, 384], dt.float32, tag="ymm")
            nc.tensor.matmul(out=r2[:, 0:MS * 64], lhsT=gaT[:],
                             rhs=W2BLK[:], start=True, stop=True)
            r2b = sp.tile([128, MS * 64], dt.bfloat16, tag="r2b")
            nc.vector.tensor_tensor(out=r2b[:], in0=r2[:, 0:MS * 64],
                                    in1=B2REP[:], op=Alu.add)
            r2r = sp.tile([128, MS * 64], dt.bfloat16, tag="r2r")
            nc.scalar.activation(out=r2r[:], in_=r2b[:], func=Act.Relu,
                                 bias=BIASV[:, 4:5])
            pz = sp.tile([128, MS * 64], dt.bfloat16, tag="pz")
            nc.vector.tensor_tensor(out=pz[:], in0=r2r[:], in1=WCDREP[:],
                                    op=Alu.mult)
            zd = sp.tile([128, MS], dt.float32, tag="zd")
            nc.vector.tensor_reduce(
                out=zd[:], in_=pz[:].rearrange("p (s o) -> p s o", s=MS),
                axis=Ax.X, op=Alu.add)
            probs = sp.tile([128, MS, 2], dt.bfloat16, tag="probs")
            nc.scalar.activation(
                out=probs[:, :, 1:2].rearrange("p s a -> p (s a)"),
                in_=zd[:], func=Act.Sigmoid, bias=BIASV[:, 2:3])
            nc.scalar.activation(
                out=probs[:, :, 0:1].rearrange("p s a -> p (s a)"),
                in_=zd[:], func=Act.Sigmoid, bias=BIASV[:, 3:4],
                scale=-1.0)
            accm = ps_mm.tile([2, 8], dt.float32, tag="ymm")
            for s in range(MS):
                nc.tensor.matmul(
                    out=accm[:],
                    lhsT=probs[:, s:s + 1, :].rearrange("p a c -> p (a c)"),
                    rhs=sbt[:, s:s + 1, :].rearrange("p a c -> p (a c)"),
                    start=(s == 0), stop=(s == MS - 1))
            nc.vector.tensor_tensor(out=acc_sb[:], in0=acc_sb[:],
                                    in1=accm[:], op=Alu.add)

        # --- software pipeline: tok0 first, gathers chain, scalar is tail-only ---
        states = {0: load_dmas(0)}
        encode(states[0])
        gather(states[0])
        for m in range(1, MACROS):
            states[m] = load_dmas(m)
            encode(states[m])
        for m in range(MACROS):
            compute_head(states[m], m)
            if m + 1 < MACROS:
                gather(states[m + 1])
            if m >= 1:
                tail_conv(states[m - 1], m - 1)
            if m >= 2:
                tail_rest(states[m - 2], m - 2)
                del states[m - 2]
        tail_conv(states[MACROS - 1], MACROS - 1)
        tail_rest(states[MACROS - 2], MACROS - 2)
        tail_rest(states[MACROS - 1], MACROS - 1)

        outs = sp.tile([2, 8], dt.float32, tag="outs")
        nc.scalar.mul(out=outs[:], in_=acc_sb[:], mul=1.0 / N)
        nc.sync.dma_start(out_d.rearrange("b c -> c b"), outs[:])

    nc.compile()
    return nc


# ----------------------------------------------------------------------------
# v1 fallback (exact 16x gather) -- original kernel, kept verbatim
# ----------------------------------------------------------------------------

def _conv_chunk_weights(wf):
    """Block-Toeplitz per chunk: rows f-major (f, delta), cols (o_local, ch)."""
    Ws = []
    for (start, opc) in CHUNKS_V1:
        span = opc + KW - 1
        W = np.zeros((3 * span, opc * 12), np.float32)
        for f in range(3):
            for d in range(span):
                for o in range(opc):
                    k = d - o
                    if 0 <= k < KW:
                        W[f * span + d, o * 12:(o + 1) * 12] = wf[k, f, :]
        Ws.append(W)
    return Ws


def _d1_weights(w1):
    RA = np.zeros((72, 192), np.float32)
    RB = np.zeros((60, 160), np.float32)
    for p in range(6):
        RA[12 * p:12 * p + 12, 32 * p:32 * p + 32] = w1
    for p in range(5):
        RB[12 * p:12 * p + 12, 32 * p:32 * p + 32] = w1
    return RA, RB


def _build_host_data_v1(tokens, lengths, embed_w, conv_w, conv_b, w1, b1, w2, b2,
                     wc, bc):
    wf, bconv = _fold_weights(embed_w, conv_w, conv_b)
    Ws = _conv_chunk_weights(wf)
    RA, RB = _d1_weights(w1)
    wcd = (wc[:, 1] - wc[:, 0]).astype(np.float32)
    bcd = float(bc[1] - bc[0])

    consts = {
        "w_c0": Ws[0], "w_c1": Ws[1], "w_c2": Ws[2], "w_c3": Ws[3],
        "r1a": RA, "r1b": RB,
        "w2r": w2.astype(np.float32),
        "maskc": np.stack([(np.arange(16) == (p % 16)) for p in range(128)]
                          ).astype(np.float32),                  # [128,16]
        "ident": np.eye(128, dtype=np.float32),
        "bpool": np.tile(np.tile(bconv, NPOOL_V1)[None, :], (128, 1)),  # [128,132]
        "b1rep": np.tile(b1.astype(np.float32)[None, :], (128, 1)),
        "b2rep": np.tile(b2.astype(np.float32)[None, :], (128, 1)),
        "wcdrep": np.tile(wcd[None, :], (128, 1)),
        "ones1": np.ones((128, 1), np.float32),
        "biasv": np.tile(np.array([-2.0, -3.0, bcd, -bcd, 0.0, 0.0, 0.0, 0.0],
                                  np.float32)[None, :], (128, 1)),
    }

    tokens_r = tokens.reshape(N_CORES, SEQ_PER_CORE, L)
    lengths_r = lengths.reshape(N_CORES, SEQ_PER_CORE)
    per_core = []
    for c in range(N_CORES):
        lo, fr = _resize_tables(lengths_r[c])
        m = dict(consts)
        m["tok"] = np.ascontiguousarray(tokens_r[c])
        m["idx"] = lo.astype(np.uint16).reshape(TILES_V1, 128, TARGET)
        m["frac"] = fr.reshape(TILES_V1, 128, TARGET)
        per_core.append(m)
    return per_core, bcd


# ----------------------------------------------------------------------------
# device program
# ----------------------------------------------------------------------------

def _build_program_v1(bcd, repeat=1, ablate=()):
    import concourse.bass as bass
    import concourse.tile as tile
    from concourse import bacc, mybir

    dt = mybir.dt
    Alu = mybir.AluOpType
    Act = mybir.ActivationFunctionType
    Ax = mybir.AxisListType

    nc = bacc.Bacc("TRN2", target_bir_lowering=False, debug=False)

    def din(name, shape, dtype):
        return nc.dram_tensor(name, shape, dtype, kind="ExternalInput").ap()

    tok_d = din("tok", [SEQ_PER_CORE, L], dt.int32)
    idx_d = din("idx", [TILES_V1, 128, TARGET], dt.uint16)
    frac_d = din("frac", [TILES_V1, 128, TARGET], dt.float32)
    wc_d = [din(f"w_c{i}", [3 * (opc + 7), opc * 12], dt.float32)
            for i, (_, opc) in enumerate(CHUNKS_V1)]
    r1a_d = din("r1a", [72, 192], dt.float32)
    r1b_d = din("r1b", [60, 160], dt.float32)
    w2r_d = din("w2r", [32, 64], dt.float32)
    maskc_d = din("maskc", [128, 16], dt.float32)
    ident_d = din("ident", [128, 128], dt.float32)
    bpool_d = din("bpool", [128, 132], dt.float32)
    b1rep_d = din("b1rep", [128, 32], dt.float32)
    b2rep_d = din("b2rep", [128, 64], dt.float32)
    wcdrep_d = din("wcdrep", [128, 64], dt.float32)
    ones1_d = din("ones1", [128, 1], dt.float32)
    biasv_d = din("biasv", [128, 8], dt.float32)
    out_d = nc.dram_tensor("out", [B // N_CORES, 2], dt.float32,
                           kind="ExternalOutput").ap()

    with tile.TileContext(nc) as tc, ExitStack() as ctx:
        cpool = ctx.enter_context(tc.tile_pool(name="consts", bufs=1))
        iopool = ctx.enter_context(tc.tile_pool(name="io", bufs=2))
        gpool = ctx.enter_context(tc.tile_pool(name="gather", bufs=2))
        wpool = ctx.enter_context(tc.tile_pool(name="work", bufs=2))
        pspool = ctx.enter_context(tc.tile_pool(name="ps", bufs=2, space="PSUM"))
        accpool = ctx.enter_context(tc.tile_pool(name="acc", bufs=1, space="PSUM"))

        def cload(ap_d, shape, dtype, tag):
            t = cpool.tile(shape, dtype, tag=tag)
            nc.sync.dma_start(t[:], ap_d[:])
            return t

        Wc = [cload(wc_d[i], [3 * (opc + 7), opc * 12], dt.float32, f"cW{i}")
              for i, (_, opc) in enumerate(CHUNKS_V1)]
        R1A = cload(r1a_d, [72, 192], dt.float32, "cR1A")
        R1B = cload(r1b_d, [60, 160], dt.float32, "cR1B")
        W2R = cload(w2r_d, [32, 64], dt.float32, "cW2R")
        MASKC = cload(maskc_d, [128, 16], dt.float32, "cMASK")
        IDENT = cload(ident_d, [128, 128], dt.float32, "cID")
        BPOOL = cload(bpool_d, [128, 132], dt.float32, "cBP")
        B1REP = cload(b1rep_d, [128, 32], dt.float32, "cB1")
        B2REP = cload(b2rep_d, [128, 64], dt.float32, "cB2")
        WCDREP = cload(wcdrep_d, [128, 64], dt.float32, "cWCD")
        ONES1 = cload(ones1_d, [128, 1], dt.float32, "cON")
        BIASV = cload(biasv_d, [128, 8], dt.float32, "cBV")

        mean_ps = accpool.tile([2, TILES_V1], dt.float32)

        rep_ctx = tc.For_i(0, repeat, 1) if repeat > 1 else None
        if rep_ctx is not None:
            rep_ctx.__enter__()
        for t in range(TILES_V1):
            # ---- load tile inputs (tokens cast int32->fp32 via SWDGE) ----
            data3 = iopool.tile([128, L // 2, 2], dt.float32, tag="data3")
            nc.gpsimd.dma_start(
                data3[:],
                tok_d[t * 128:(t + 1) * 128, :].rearrange("p (n d) -> p n d", d=2))
            idxt = iopool.tile([128, TARGET], dt.uint16, tag="idxt")
            nc.sync.dma_start(idxt[:], idx_d[t])
            frct = iopool.tile([128, TARGET], dt.float32, tag="frct")
            nc.sync.dma_start(frct[:], frac_d[t])

            # ---- gather pairs: junk[p, i*16+k, e] = tok[p, lo[seq16k, i]+e] ----
            junk = gpool.tile([128, TARGET * 16, 2], dt.float32, tag="junk")
            if "gather" in ablate:
                nc.vector.memset(junk[:].rearrange("p a b -> p (a b)"), 1.0)
            else:
                for (i0, ni) in ((0, 32), (32, 32), (64, 32), (96, 4)):
                    nc.gpsimd.indirect_copy(
                        junk[:, i0 * 16:(i0 + ni) * 16, :],
                        data3[:],
                        idxt[:, i0:i0 + ni],
                        i_know_ap_gather_is_preferred=True)

            # ---- extract tlo/thi via masked grouped reduce ----
            jv = junk[:].rearrange("p (i k) e -> p i k e", k=16)
            mb = MASKC[:].rearrange("p (a k) -> p a k", a=1) \
                         .to_broadcast([128, TARGET, 16])
            prod = wpool.tile([128, TARGET, 16], dt.float32, tag="prod")
            lo3 = wpool.tile([128, 3, TARGET], dt.float32, tag="lo3")
            hi3 = wpool.tile([128, 3, TARGET], dt.float32, tag="hi3")
            nc.vector.tensor_tensor(out=prod[:], in0=jv[:, :, :, 0], in1=mb,
                                    op=Alu.mult)
            nc.vector.tensor_reduce(out=lo3[:, 0, :], in_=prod[:], axis=Ax.X,
                                    op=Alu.add)
            nc.vector.tensor_tensor(out=prod[:], in0=jv[:, :, :, 1], in1=mb,
                                    op=Alu.mult)
            nc.vector.tensor_reduce(out=hi3[:, 0, :], in_=prod[:], axis=Ax.X,
                                    op=Alu.add)

            # ---- features: f1 = t, f2 = relu(t-2), f3 = relu(t-3) ----
            for buf in (lo3, hi3):
                nc.scalar.activation(out=buf[:, 1, :], in_=buf[:, 0, :],
                                     func=Act.Relu, bias=BIASV[:, 0:1])
                nc.scalar.activation(out=buf[:, 2, :], in_=buf[:, 0, :],
                                     func=Act.Relu, bias=BIASV[:, 1:2])

            # ---- bilinear blend: d3 = frac*(hi3 - lo3); x3c per chunk ----
            d3 = wpool.tile([128, 3, TARGET], dt.float32, tag="d3")
            frb = frct[:].rearrange("p (a i) -> p a i", a=1) \
                         .to_broadcast([128, 3, TARGET])
            nc.vector.tensor_tensor(out=d3[:], in0=hi3[:], in1=lo3[:],
                                    op=Alu.subtract)
            nc.vector.tensor_tensor(out=d3[:], in0=d3[:], in1=frb, op=Alu.mult)

            # ---- conv: chunk-contiguous blend -> transpose -> matmul -> pool ----
            pooled = wpool.tile([128, 132], dt.float32, tag="pooled")
            for ci, (start, opc) in enumerate(CHUNKS_V1):
                span = opc + 7
                x3c = wpool.tile([128, 3, span], dt.float32, tag="x3c")
                nc.vector.tensor_tensor(out=x3c[:],
                                        in0=lo3[:, :, start:start + span],
                                        in1=d3[:, :, start:start + span],
                                        op=Alu.add)
                tp = pspool.tile([3 * 31, 128], dt.float32, tag="tp")
                nc.tensor.transpose(
                    out=tp[:3 * span, :],
                    in_=x3c[:].rearrange("p f s -> p (f s)"),
                    identity=IDENT[:])
                xtc = wpool.tile([3 * 31, 128], dt.float32, tag="xtc")
                nc.scalar.copy(out=xtc[:3 * span, :], in_=tp[:3 * span, :])
                y_ps = pspool.tile([128, opc * 12], dt.float32, tag="mm")
                nc.tensor.matmul(out=y_ps[:], lhsT=xtc[:3 * span, :],
                                 rhs=Wc[ci][:], start=True, stop=True)
                g = opc // 8
                yv = y_ps[:].rearrange("p (g o c) -> p g c o", g=g, o=8)
                nc.vector.tensor_reduce(
                    out=pooled[:, 36 * ci:36 * ci + 12 * g]
                        .rearrange("p (g c) -> p g c", g=g),
                    in_=yv, axis=Ax.X, op=Alu.max)

            # ---- bias + relu ----
            h = wpool.tile([128, 132], dt.float32, tag="h")
            nc.vector.tensor_tensor(out=h[:], in0=pooled[:], in1=BPOOL[:],
                                    op=Alu.add)
            nc.scalar.activation(out=h[:], in_=h[:], func=Act.Relu,
                                 bias=BIASV[:, 4:5])

            # ---- dense1 (block-diag) + global max over 11 pools ----
            htA_ps = pspool.tile([72, 128], dt.float32, tag="tp")
            nc.tensor.transpose(out=htA_ps[:], in_=h[:, 0:72], identity=IDENT[:])
            htA = wpool.tile([72, 128], dt.float32, tag="htA")
            nc.scalar.copy(out=htA[:], in_=htA_ps[:])
            htB_ps = pspool.tile([60, 128], dt.float32, tag="tp")
            nc.tensor.transpose(out=htB_ps[:], in_=h[:, 72:132], identity=IDENT[:])
            htB = wpool.tile([60, 128], dt.float32, tag="htB")
            nc.scalar.copy(out=htB[:], in_=htB_ps[:])

            h1a_ps = pspool.tile([128, 192], dt.float32, tag="mm")
            nc.tensor.matmul(out=h1a_ps[:], lhsT=htA[:], rhs=R1A[:],
                             start=True, stop=True)
            h1b_ps = pspool.tile([128, 160], dt.float32, tag="mm2")
            nc.tensor.matmul(out=h1b_ps[:], lhsT=htB[:], rhs=R1B[:],
                             start=True, stop=True)

            ga = wpool.tile([128, 32], dt.float32, tag="ga")
            gb = wpool.tile([128, 32], dt.float32, tag="gb")
            nc.vector.tensor_reduce(
                out=ga[:], in_=h1a_ps[:].rearrange("p (g o) -> p o g", g=6),
                axis=Ax.X, op=Alu.max)
            nc.vector.tensor_reduce(
                out=gb[:], in_=h1b_ps[:].rearrange("p (g o) -> p o g", g=5),
                axis=Ax.X, op=Alu.max)
            nc.vector.tensor_tensor(out=ga[:], in0=ga[:], in1=gb[:], op=Alu.max)
            nc.vector.tensor_tensor(out=ga[:], in0=ga[:], in1=B1REP[:], op=Alu.add)
            nc.scalar.activation(out=ga[:], in_=ga[:], func=Act.Relu,
                                 bias=BIASV[:, 4:5])

            # ---- dense2 + relu ----
            gt_ps = pspool.tile([32, 128], dt.float32, tag="tp")
            nc.tensor.transpose(out=gt_ps[:], in_=ga[:], identity=IDENT[:])
            gt = wpool.tile([32, 128], dt.float32, tag="gt")
            nc.scalar.copy(out=gt[:], in_=gt_ps[:])
            r2_ps = pspool.tile([128, 64], dt.float32, tag="mm2")
            nc.tensor.matmul(out=r2_ps[:], lhsT=gt[:], rhs=W2R[:],
                             start=True, stop=True)
            r2 = wpool.tile([128, 64], dt.float32, tag="r2")
            nc.vector.tensor_tensor(out=r2[:], in0=r2_ps[:], in1=B2REP[:],
                                    op=Alu.add)
            nc.scalar.activation(out=r2[:], in_=r2[:], func=Act.Relu,
                                 bias=BIASV[:, 4:5])

            # ---- classifier: zd = r2@wcd + bcd; probs = [sig(-zd), sig(zd)] ----
            pz = wpool.tile([128, 64], dt.float32, tag="pz")
            nc.vector.tensor_tensor(out=pz[:], in0=r2[:], in1=WCDREP[:],
                                    op=Alu.mult)
            zd = wpool.tile([128, 1], dt.float32, tag="zd")
            nc.vector.tensor_reduce(out=zd[:], in_=pz[:], axis=Ax.X, op=Alu.add)
            probs = wpool.tile([128, 2], dt.float32, tag="probs")
            nc.scalar.activation(out=probs[:, 1:2], in_=zd[:], func=Act.Sigmoid,
                                 bias=BIASV[:, 2:3])
            nc.scalar.activation(out=probs[:, 0:1], in_=zd[:], func=Act.Sigmoid,
                                 bias=BIASV[:, 3:4], scale=-1.0)

            # ---- per-tile node-sum: mean_ps[:, t] = probs.T @ ones ----
            nc.tensor.matmul(out=mean_ps[:, t:t + 1], lhsT=probs[:],
                             rhs=ONES1[:], start=True, stop=True)

        if rep_ctx is not None:
            rep_ctx.__exit__(None, None, None)

        # ---- finalize: combine tile pairs, /256, write [8, 2] ----
        sums = wpool.tile([2, B // N_CORES], dt.float32, tag="sums")
        nc.vector.tensor_reduce(
            out=sums[:], in_=mean_ps[:].rearrange("p (b t) -> p b t", t=2),
            axis=Ax.X, op=Alu.add)
        outs = wpool.tile([2, B // N_CORES], dt.float32, tag="outs")
        nc.scalar.mul(out=outs[:], in_=sums[:], mul=1.0 / N)
        nc.sync.dma_start(out_d.rearrange("b c -> c b"), outs[:])

    nc.compile()
    return nc




# ----------------------------------------------------------------------------
# entry point
# ----------------------------------------------------------------------------

def _run(nc, per_core):
    from concourse.bass_utils import run_bass_kernel_spmd
    global LAST_EXEC_NS, LAST_TRACE_PATH
    if TRACE:
        res = run_bass_kernel_spmd(nc, per_core, list(range(N_CORES)),
                                   trace=True, tmpdir=TRACE_DIR)
        LAST_EXEC_NS = res.exec_time_ns
        if res.instructions_and_trace is not None:
            LAST_TRACE_PATH = res.instructions_and_trace[1]
    else:
        res = run_bass_kernel_spmd(nc, per_core, list(range(N_CORES)))
    return np.concatenate([res.results[c]["out"] for c in range(N_CORES)],
                          axis=0).astype(np.float32)


def kernel(**inputs):
    tokens = np.asarray(inputs["tokens"])
    lengths = np.asarray(inputs["lengths"])
    args = [np.asarray(inputs[k]) for k in
            ("embed_w", "conv_w", "conv_b", "w1", "b1", "w2", "b2", "wc", "bc")]

    consts, bcd = build_consts(*args)
    tokens_r = tokens.reshape(N_CORES, SEQ_PER_CORE, L)
    lengths_r = lengths.reshape(N_CORES, SEQ_PER_CORE)
    per_core = []
    ok = True
    for c in range(N_CORES):
        md, mo = plan_core(tokens_r[c], lengths_r[c])
        if mo > W - 1:
            ok = False
            break
        d = dict(consts)
        d.update(md)
        per_core.append(d)

    if ok:
        key = ("v2", round(bcd, 8))
        if key not in _CACHE:
            _CACHE[key] = build_program(bcd)
        return _run(_CACHE[key], per_core)

    # fallback: exact-gather v1 program
    per_core_v1, bcd = _build_host_data_v1(tokens, lengths, *args)
    key = ("v1", round(bcd, 8))
    if key not in _CACHE:
        _CACHE[key] = _build_program_v1(bcd)
    return _run(_CACHE[key], per_core_v1)


# revision 5
# speedup vs baseline: 1.0127x; 1.0127x over previous
"""Trainium2 Bass kernel for nn_CNNModel (ragged resize -> conv1d -> pools -> MLP).

Self-contained: hardcodes shapes B=64, N=256, L=1024, TARGET=100, 8 cores.
Pure data parallel over batch (2048 seqs/core).

v2 path: host sorts seqs by length so 16-seq gather groups share W=16 windows
on pair-encoded tokens u[l] = t[l] + 8*t[l+1]; 4 macro-tiles of 512 seqs.
Device: SWDGE int32->bf16 token DMA -> u-encode (DVE stt) -> gpsimd
indirect_copy window gathers (d=8 int32 view) -> DMA'd one-hot extract +
tree-reduce -> is_ge decode -> phi features + frac blend (all DVE) -> 4-chunk
block-Toeplitz conv matmuls (bias via ones channel) -> relu-copy + tree
maxpool -> block-diag dense1 + global max -> block-diag dense2 -> sigmoid
classifier -> per-macro PSUM S-matmul batch attribution; 3-phase software
pipeline (head / tail_conv / tail_rest) across macros.

Fallback (v1, exact 16x-redundant gather) if any gather group's window
offsets exceed [0, 15] (host-verified per run).
"""

from contextlib import ExitStack

import numpy as np

B, N, L = 64, 256, 1024
TARGET = 100
KW = 8
N_CORES = 8
SEQ_PER_CORE = 2048
MS = 4
MACROS = 4
W = 16
CHUNKS = [(0, 24), (24, 24), (48, 24), (72, 16)]
POFF = [0, 3, 6, 9]
NW = MS * TARGET

# v1 fallback constants
NPOOL_V1 = 11
TILES_V1 = SEQ_PER_CORE // 128
CHUNKS_V1 = [(0, 24), (24, 24), (48, 24), (72, 16)]

_CACHE = {}

# set by test.py to capture an NTFF profile; harness leaves these alone
TRACE = False
TRACE_DIR = None
LAST_EXEC_NS = None
LAST_TRACE_PATH = None


def _bf16():
    import ml_dtypes
    return ml_dtypes.bfloat16


# ----------------------------------------------------------------------------
# host-side
# ----------------------------------------------------------------------------

def _resize_tables(lengths_flat):
    lf = lengths_flat.astype(np.float64)[:, None]
    i = np.arange(TARGET, dtype=np.float64)[None, :]
    src = (i + 0.5) * lf / TARGET - 0.5
    f = np.floor(src)
    lo = np.clip(f, 0, lf - 2).astype(np.int64)
    fr = np.clip(src - lo, 0.0, 1.0)
    return lo, fr.astype(np.float32)


def _fold_weights(embed_w, conv_w, conv_b):
    Phi = np.array([[1, 1, 0, 0], [1, 2, 0, 0], [1, 3, 1, 0], [1, 4, 2, 1]],
                   dtype=np.float64)
    E = embed_w[1:5].astype(np.float64)
    M = np.linalg.solve(Phi, E)
    wf = np.einsum("fc,kco->kfo", M[1:], conv_w.astype(np.float64))
    bias = conv_b.astype(np.float64) + np.einsum("c,kco->o", M[0],
                                                 conv_w.astype(np.float64))
    return wf.astype(np.float32), bias.astype(np.float32)


def build_consts(embed_w, conv_w, conv_b, w1, b1, w2, b2, wc, bc):
    bf16 = _bf16()
    wf, bconv = _fold_weights(embed_w, conv_w, conv_b)
    Wcs = []
    for (st, opc) in CHUNKS:
        span = opc + KW - 1
        Wm = np.zeros((4 * span, opc * 12), np.float32)
        for dlt in range(span):
            for o in range(opc):
                k = dlt - o
                if 0 <= k < KW:
                    for f in range(3):
                        Wm[dlt * 4 + f, o * 12:(o + 1) * 12] = wf[k, f, :]
                if k == 0:
                    Wm[dlt * 4 + 3, o * 12:(o + 1) * 12] = bconv
        Wcs.append(Wm.astype(bf16))
    R1A = np.zeros((72, 192), np.float32)
    R1B = np.zeros((60, 160), np.float32)
    for p in range(6):
        R1A[12 * p:12 * p + 12, 32 * p:32 * p + 32] = w1
    for p in range(5):
        R1B[12 * p:12 * p + 12, 32 * p:32 * p + 32] = w1
    W2blk = np.zeros((128, MS * 64), np.float32)
    for s in range(MS):
        W2blk[s * 32:(s + 1) * 32, s * 64:(s + 1) * 64] = w2
    wcd = (wc[:, 1] - wc[:, 0]).astype(np.float32)
    bcd = float(bc[1] - bc[0])
    consts = {
        "r1a": R1A.astype(bf16), "r1b": R1B.astype(bf16),
        "w2blk": W2blk.astype(bf16),
        "b1rep": np.tile(b1.astype(np.float32), MS)[None, :]
                 .repeat(128, 0).astype(bf16),
        "b2rep": np.tile(b2.astype(np.float32), MS)[None, :]
                 .repeat(128, 0).astype(bf16),
        "wcdrep": np.tile(wcd, MS)[None, :].repeat(128, 0).astype(bf16),
        "iota16": np.arange(16, dtype=np.float32)[None, :]
                  .repeat(128, 0).astype(bf16),
        "identb": np.eye(128, dtype=np.float32).astype(bf16),
        "biasv": np.tile(np.array([-2.0, -3.0, bcd, -bcd, 0.0, 0.0, 0.0, 0.0],
                                  np.float32)[None, :], (128, 1)),
    }
    for i, W in enumerate(Wcs):
        consts[f"w_c{i}"] = W
    return consts, bcd


def plan_core(tokens_core, lengths_core):
    """-> (dict of per-macro arrays, max_o). Window bases are even (int32
    gather units); o = lo - base must be in [0, 15]."""
    bf16 = _bf16()
    lo, fr = _resize_tables(lengths_core)
    perm = np.argsort(lengths_core, kind="stable")
    out = {}
    max_o = 0
    jj = np.arange(NW)
    ss, ii = jj // TARGET, jj % TARGET
    for m in range(MACROS):
        sm = np.zeros((128, MS), np.int64)
        for g in range(8):
            for s in range(MS):
                c = m * 32 + g * 4 + s
                sm[16 * g:16 * g + 16, s] = perm[16 * c:16 * c + 16]
        lo_m = lo[sm]                                  # [128, MS, 100]
        fr_m = fr[sm]
        lo_g = lo_m.reshape(8, 16, MS, TARGET)
        lomin = lo_g.min(axis=1)                       # [8, MS, 100]
        base = np.minimum(lomin - (lomin % 2), 1008)
        o = lo_g - base[:, None]                       # [8, 16, MS, 100]
        max_o = max(max_o, int(o.max()))
        idx = np.zeros((128, 28), np.uint16)
        j25 = np.arange(25)
        for g in range(8):
            for k in range(16):
                j = j25 * 16 + k
                idx[16 * g + k, :25] = \
                    (ss[j] * L + base[g, ss[j], ii[j]]) // 2
        out[f"tok{m}"] = np.ascontiguousarray(
            tokens_core[sm].reshape(128, MS * L))
        out[f"idx{m}"] = idx
        ot = np.ascontiguousarray(
            o.transpose(0, 1, 2, 3).reshape(128, NW))
        out[f"oh{m}"] = (ot[:, :, None] ==
                         np.arange(W)[None, None, :]).astype(bf16)
        out[f"fr{m}"] = np.ascontiguousarray(
            fr_m.reshape(128, NW)).astype(bf16)
        S = np.zeros((128, MS, 8), np.float32)
        bidx = sm // N
        for s in range(MS):
            S[np.arange(128), s, bidx[:, s]] = 1.0
        out[f"sb{m}"] = S.astype(bf16)
    return out, max_o


# ----------------------------------------------------------------------------
# device program
# ----------------------------------------------------------------------------

def build_program(bcd):
    import concourse.bass as bass  # noqa: F401
    import concourse.tile as tile
    from concourse import bacc, mybir

    dt = mybir.dt
    Alu = mybir.AluOpType
    Act = mybir.ActivationFunctionType
    Ax = mybir.AxisListType

    nc = bacc.Bacc("TRN2", target_bir_lowering=False, debug=False)

    def din(name, shape, dtype):
        return nc.dram_tensor(name, shape, dtype, kind="ExternalInput").ap()

    tok_d = [din(f"tok{m}", [128, MS * L], dt.int32) for m in range(MACROS)]
    idx_d = [din(f"idx{m}", [128, 28], dt.uint16) for m in range(MACROS)]
    oh_d = [din(f"oh{m}", [128, NW, W], dt.bfloat16) for m in range(MACROS)]
    fr_d = [din(f"fr{m}", [128, NW], dt.bfloat16) for m in range(MACROS)]
    sb_d = [din(f"sb{m}", [128, MS, 8], dt.bfloat16) for m in range(MACROS)]
    wc_d = [din(f"w_c{i}", [4 * (opc + 7), opc * 12], dt.bfloat16)
            for i, (_, opc) in enumerate(CHUNKS)]
    r1a_d = din("r1a", [72, 192], dt.bfloat16)
    r1b_d = din("r1b", [60, 160], dt.bfloat16)
    b1rep_d = din("b1rep", [128, MS * 32], dt.bfloat16)
    w2blk_d = din("w2blk", [128, MS * 64], dt.bfloat16)
    b2rep_d = din("b2rep", [128, MS * 64], dt.bfloat16)
    wcdrep_d = din("wcdrep", [128, MS * 64], dt.bfloat16)
    identb_d = din("identb", [128, 128], dt.bfloat16)
    biasv_d = din("biasv", [128, 8], dt.float32)
    out_d = nc.dram_tensor("out", [B // N_CORES, 2], dt.float32,
                           kind="ExternalOutput").ap()

    with tile.TileContext(nc) as tc, ExitStack() as ctx:
        cpool = ctx.enter_context(tc.tile_pool(name="consts", bufs=1))
        iop = ctx.enter_context(tc.tile_pool(name="io", bufs=4))
        auxp = ctx.enter_context(tc.tile_pool(name="aux", bufs=4))
        ohp = ctx.enter_context(tc.tile_pool(name="ohp", bufs=2))
        lhp = ctx.enter_context(tc.tile_pool(name="lhs", bufs=3))
        wp = ctx.enter_context(tc.tile_pool(name="work", bufs=2))
        sp = ctx.enter_context(tc.tile_pool(name="single", bufs=1))
        ps_tp = ctx.enter_context(tc.tile_pool(name="ps_tp", bufs=3,
                                               space="PSUM"))
        ps_mm = ctx.enter_context(tc.tile_pool(name="ps_mm", bufs=3,
                                               space="PSUM"))
        ps_h1 = ctx.enter_context(tc.tile_pool(name="ps_h1", bufs=2,
                                               space="PSUM"))

        def cload(ap_d, shape, dtype, tag):
            t = cpool.tile(shape, dtype, tag=tag)
            nc.sync.dma_start(t[:], ap_d[:])
            return t

        Wc = [cload(wc_d[i], [4 * (opc + 7), opc * 12], dt.bfloat16,
                    f"cW{i}") for i, (_, opc) in enumerate(CHUNKS)]
        R1A = cload(r1a_d, [72, 192], dt.bfloat16, "cR1A")
        R1B = cload(r1b_d, [60, 160], dt.bfloat16, "cR1B")
        B1REP = cload(b1rep_d, [128, MS * 32], dt.bfloat16, "cB1")
        W2BLK = cload(w2blk_d, [128, MS * 64], dt.bfloat16, "cW2B")
        B2REP = cload(b2rep_d, [128, MS * 64], dt.bfloat16, "cB2")
        WCDREP = cload(wcdrep_d, [128, MS * 64], dt.bfloat16, "cWCD")
        IDENT = cload(identb_d, [128, 128], dt.bfloat16, "cID")
        BIASV = cload(biasv_d, [128, 8], dt.float32, "cBV")
        ONE400 = cpool.tile([128, NW], dt.bfloat16, tag="cONE")
        nc.vector.memset(ONE400[:], 1.0)

        acc_sb = cpool.tile([2, 8], dt.float32, tag="accsb")
        nc.vector.memset(acc_sb[:], 0.0)

        def load_dmas(m):
            tokb = iop.tile([128, MS * L], dt.bfloat16, tag="tokb")
            nc.gpsimd.dma_start(tokb[:], tok_d[m][:])
            idxt = auxp.tile([128, 28], dt.uint16, tag="idxt")
            nc.scalar.dma_start(idxt[:], idx_d[m][:])
            frt = auxp.tile([128, NW], dt.bfloat16, tag="frt")
            nc.scalar.dma_start(frt[:], fr_d[m][:])
            sbt = auxp.tile([128, MS, 8], dt.bfloat16, tag="sbt")
            nc.scalar.dma_start(sbt[:], sb_d[m][:])
            oht = ohp.tile([128, NW, W], dt.bfloat16, tag="oht")
            nc.sync.dma_start(oht[:], oh_d[m][:])
            return dict(tokb=tokb, idxt=idxt, oht=oht, frt=frt, sbt=sbt)

        def encode(st):
            tokb = st["tokb"]
            u = iop.tile([128, MS * L], dt.bfloat16, tag="u")
            nv = MS * L - 1
            # u[l] = 8*t[l+1] + t[l]  (one DVE scalar_tensor_tensor)
            nc.vector.memset(u[:, nv:nv + 1], 9.0)
            nc.vector.scalar_tensor_tensor(out=u[:, 0:nv],
                                           in0=tokb[:, 1:MS * L], scalar=8.0,
                                           in1=tokb[:, 0:nv], op0=Alu.mult,
                                           op1=Alu.add)
            st["u"] = u

        def gather(st):
            win = wp.tile([128, 448, W], dt.bfloat16, tag="win")
            u32 = st["u"][:].bitcast(dt.int32).rearrange(
                "p (n d) -> p n d", d=W // 2)
            w32 = win[:].bitcast(dt.int32)
            for j in range(7):
                nc.gpsimd.indirect_copy(
                    w32[:, 64 * j:64 * j + 64, :], u32,
                    st["idxt"][:, 4 * j:4 * j + 4],
                    i_know_ap_gather_is_preferred=True)
            st["win"] = win
            return st

        def compute_head(st, m):
            win, oh, frt = st["win"], st["oht"], st["frt"]
            prod = sp.tile([128, NW, W], dt.bfloat16, tag="prod")
            nc.vector.tensor_tensor(out=prod[:], in0=win[:, 0:NW, :],
                                    in1=oh[:], op=Alu.mult)
            t1 = sp.tile([128, NW, 8], dt.bfloat16, tag="t1")
            nc.vector.tensor_tensor(out=t1[:], in0=prod[:, :, 0:8],
                                    in1=prod[:, :, 8:16], op=Alu.add)
            t2 = sp.tile([128, NW, 4], dt.bfloat16, tag="t2")
            nc.vector.tensor_tensor(out=t2[:], in0=t1[:, :, 0:4],
                                    in1=t1[:, :, 4:8], op=Alu.add)
            t3 = sp.tile([128, NW, 2], dt.bfloat16, tag="t3")
            nc.vector.tensor_tensor(out=t3[:], in0=t2[:, :, 0:2],
                                    in1=t2[:, :, 2:4], op=Alu.add)
            usel = sp.tile([128, NW], dt.bfloat16, tag="usel")
            nc.vector.tensor_tensor(
                out=usel[:].rearrange("p (i a) -> p i a", a=1),
                in0=t3[:, :, 0:1], in1=t3[:, :, 1:2], op=Alu.add)
            # --- decode u = t_lo + 8*t_hi (phi tiles are x-major [p, NW, 3]) ---
            phiL = sp.tile([128, NW, 3], dt.bfloat16, tag="phiL")
            phiH = sp.tile([128, NW, 3], dt.bfloat16, tag="phiH")
            tlv = phiL[:, :, 0:1].rearrange("p a b -> p (a b)")
            thv = phiH[:, :, 0:1].rearrange("p a b -> p (a b)")
            dA = sp.tile([128, NW], dt.bfloat16, tag="dA")
            nc.vector.scalar_tensor_tensor(out=dA[:], in0=usel[:],
                                           scalar=32.0, in1=ONE400[:],
                                           op0=Alu.is_ge, op1=Alu.add)
            dB = sp.tile([128, NW], dt.bfloat16, tag="dB")
            nc.vector.scalar_tensor_tensor(out=dB[:], in0=usel[:],
                                           scalar=24.0, in1=dA[:],
                                           op0=Alu.is_ge, op1=Alu.add)
            nc.vector.scalar_tensor_tensor(out=thv, in0=usel[:],
                                           scalar=16.0, in1=dB[:],
                                           op0=Alu.is_ge, op1=Alu.add)
            nc.vector.scalar_tensor_tensor(out=tlv, in0=thv, scalar=-8.0,
                                           in1=usel[:], op0=Alu.mult,
                                           op1=Alu.add)
            # --- phi features (DVE tensor_scalar: relu(x-c)) ---
            for buf, src in ((phiL, tlv), (phiH, thv)):
                nc.vector.tensor_scalar(
                    out=buf[:, :, 1:2].rearrange("p a b -> p (a b)"),
                    in0=src, scalar1=2.0, scalar2=0.0,
                    op0=Alu.subtract, op1=Alu.max)
                nc.vector.tensor_scalar(
                    out=buf[:, :, 2:3].rearrange("p a b -> p (a b)"),
                    in0=src, scalar1=3.0, scalar2=0.0,
                    op0=Alu.subtract, op1=Alu.max)
            D = sp.tile([128, NW, 3], dt.bfloat16, tag="D")
            nc.vector.tensor_tensor(out=D[:], in0=phiH[:],
                                    in1=phiL[:, :, 0:3], op=Alu.subtract)
            D2 = sp.tile([128, NW, 3], dt.bfloat16, tag="D2")
            nc.vector.tensor_tensor(
                out=D2[:], in0=D[:],
                in1=frt[:].rearrange("p (j a) -> p j a", a=1)
                    .to_broadcast([128, NW, 3]), op=Alu.mult)
            F = wp.tile([128, NW, 4], dt.bfloat16, tag="F")
            nc.vector.memset(F[:, :, 3:4], 1.0)
            nc.vector.tensor_tensor(out=F[:, :, 0:3], in0=phiL[:, :, 0:3],
                                    in1=D2[:], op=Alu.add)
            st["F"] = F

        def tail_conv(st, m):
            F = st["F"]
            # --- conv chunks -> relu-copy ---
            ybuf = wp.tile([128, MS, 11 * 96], dt.bfloat16, tag="ybuf")
            for s in range(MS):
                for ci, (st_c, opc) in enumerate(CHUNKS):
                    span = opc + 7
                    rows = 4 * span
                    tp = ps_tp.tile([128, 128], dt.bfloat16, tag="tp")
                    nc.tensor.transpose(
                        out=tp[0:rows, :],
                        in_=F[:, s * TARGET + st_c:s * TARGET + st_c + span,
                              :].rearrange("p x f -> p (x f)"),
                        identity=IDENT[:])
                    xtc = lhp.tile([124, 128], dt.bfloat16, tag="xtc")
                    nc.scalar.copy(out=xtc[0:rows, :], in_=tp[0:rows, :])
                    y = ps_mm.tile([128, 384], dt.float32, tag="ymm")
                    nc.tensor.matmul(out=y[:, 0:opc * 12],
                                     lhsT=xtc[0:rows, :],
                                     rhs=Wc[ci][:], start=True, stop=True)
                    nc.scalar.activation(
                        out=ybuf[:, s:s + 1, POFF[ci] * 96:
                                 (POFF[ci] + opc // 8) * 96]
                            .rearrange("p a b -> p (a b)"),
                        in_=y[:, 0:opc * 12], func=Act.Relu,
                        bias=BIASV[:, 4:5])
            st["ybuf"] = ybuf

        def tail_rest(st, m):
            ybuf, sbt = st["ybuf"], st["sbt"]
            # maxpool(8) tree over q
            yv = ybuf[:].rearrange("p s (pl q c) -> p (s pl) q c", q=8, c=12)
            p1 = sp.tile([128, 44, 4, 12], dt.bfloat16, tag="p1")
            nc.vector.tensor_tensor(out=p1[:], in0=yv[:, :, 0:4, :],
                                    in1=yv[:, :, 4:8, :], op=Alu.max)
            p2 = sp.tile([128, 44, 2, 12], dt.bfloat16, tag="p2")
            nc.vector.tensor_tensor(out=p2[:], in0=p1[:, :, 0:2, :],
                                    in1=p1[:, :, 2:4, :], op=Alu.max)
            hr = sp.tile([128, 44, 12], dt.bfloat16, tag="hr")
            nc.vector.tensor_tensor(
                out=hr[:],
                in0=p2[:, :, 0:1, :].rearrange("p a b c -> p (a b) c"),
                in1=p2[:, :, 1:2, :].rearrange("p a b c -> p (a b) c"),
                op=Alu.max)
            # --- dense1 (block-diag) + global max ---
            gar = sp.tile([128, 128], dt.bfloat16, tag="gar")
            for s in range(MS):
                hs = hr[:, s * 11:(s + 1) * 11, :]
                tpA = ps_tp.tile([128, 128], dt.bfloat16, tag="tp")
                nc.tensor.transpose(out=tpA[0:72, :],
                                    in_=hs[:, 0:6, :], identity=IDENT[:])
                htA = sp.tile([72, 128], dt.bfloat16, tag="htA")
                nc.scalar.copy(out=htA[:], in_=tpA[0:72, :])
                tpB = ps_tp.tile([128, 128], dt.bfloat16, tag="tp")
                nc.tensor.transpose(out=tpB[0:60, :],
                                    in_=hs[:, 6:11, :], identity=IDENT[:])
                htB = sp.tile([60, 128], dt.bfloat16, tag="htB")
                nc.scalar.copy(out=htB[:], in_=tpB[0:60, :])
                h1a = ps_h1.tile([128, 192], dt.float32, tag="h1")
                nc.tensor.matmul(out=h1a[:], lhsT=htA[:], rhs=R1A[:],
                                 start=True, stop=True)
                h1b = ps_h1.tile([128, 192], dt.float32, tag="h1")
                nc.tensor.matmul(out=h1b[:, 0:160], lhsT=htB[:], rhs=R1B[:],
                                 start=True, stop=True)
                ga6 = sp.tile([128, 32], dt.bfloat16, tag="ga6")
                nc.vector.tensor_reduce(
                    out=ga6[:], in_=h1a[:].rearrange("p (g o) -> p o g", g=6),
                    axis=Ax.X, op=Alu.max)
                gb5 = sp.tile([128, 32], dt.bfloat16, tag="gb5")
                nc.vector.tensor_reduce(
                    out=gb5[:],
                    in_=h1b[:, 0:160].rearrange("p (g o) -> p o g", g=5),
                    axis=Ax.X, op=Alu.max)
                nc.vector.tensor_tensor(out=gar[:, s * 32:(s + 1) * 32],
                                        in0=ga6[:], in1=gb5[:], op=Alu.max)
            # --- dense2 (block-diag over slots) + classifier ---
            garb = sp.tile([128, 128], dt.bfloat16, tag="garb")
            nc.vector.tensor_tensor(out=garb[:], in0=gar[:], in1=B1REP[:],
                                    op=Alu.add)
            garr = sp.tile([128, 128], dt.bfloat16, tag="garr")
            nc.scalar.activation(out=garr[:], in_=garb[:], func=Act.Relu,
                                 bias=BIASV[:, 4:5])
            tpg = ps_tp.tile([128, 128], dt.bfloat16, tag="tp")
            nc.tensor.transpose(out=tpg[:], in_=garr[:], identity=IDENT[:])
            gaT = sp.tile([128, 128], dt.bfloat16, tag="gaT")
            nc.scalar.copy(out=gaT[:], in_=tpg[:])
            r2 = ps_mm.tile([128, 384], dt.float32, tag="ymm")
            nc.tensor.matmul(out=r2[:, 0:MS * 64], lhsT=gaT[:],
                             rhs=W2BLK[:], start=True, stop=True)
            r2b = sp.tile([128, MS * 64], dt.bfloat16, tag="r2b")
            nc.vector.tensor_tensor(out=r2b[:], in0=r2[:, 0:MS * 64],
                                    in1=B2REP[:], op=Alu.add)
            r2r = sp.tile([128, MS * 64], dt.bfloat16, tag="r2r")
            nc.scalar.activation(out=r2r[:], in_=r2b[:], func=Act.Relu,
                                 bias=BIASV[:, 4:5])
            pz = sp.tile([128, MS * 64], dt.bfloat16, tag="pz")
            nc.vector.tensor_tensor(out=pz[:], in0=r2r[:], in1=WCDREP[:],
                                    op=Alu.mult)
            zd = sp.tile([128, MS], dt.float32, tag="zd")
            nc.vector.tensor_reduce(
                out=zd[:], in_=pz[:].rearrange("p (s o) -> p s o", s=MS),
                axis=Ax.X, op=Alu.add)
            probs = sp.tile([128, MS, 2], dt.bfloat16, tag="probs")
            nc.scalar.activation(
                out=probs[:, :, 1:2].rearrange("p s a -> p (s a)"),
                in_=zd[:], func=Act.Sigmoid, bias=BIASV[:, 2:3])
            nc.scalar.activation(
                out=probs[:, :, 0:1].rearrange("p s a -> p (s a)"),
                in_=zd[:], func=Act.Sigmoid, bias=BIASV[:, 3:4],
                scale=-1.0)
            accm = ps_mm.tile([2, 8], dt.float32, tag="ymm")
            for s in range(MS):
                nc.tensor.matmul(
                    out=accm[:],
                    lhsT=probs[:, s:s + 1, :].rearrange("p a c -> p (a c)"),
                    rhs=sbt[:, s:s + 1, :].rearrange("p a c -> p (a c)"),
                    start=(s == 0), stop=(s == MS - 1))
            nc.vector.tensor_tensor(out=acc_sb[:], in0=acc_sb[:],
                                    in1=accm[:], op=Alu.add)

        # --- software pipeline: tok0 first, gathers chain, scalar is tail-only ---
        states = {0: load_dmas(0)}
        encode(states[0])
        gather(states[0])
        for m in range(1, MACROS):
            states[m] = load_dmas(m)
            encode(states[m])
        for m in range(MACROS):
            compute_head(states[m], m)
            if m + 1 < MACROS:
                gather(states[m + 1])
            if m >= 1:
                tail_conv(states[m - 1], m - 1)
            if m >= 2:
                tail_rest(states[m - 2], m - 2)
                del states[m - 2]
        tail_conv(states[MACROS - 1], MACROS - 1)
        tail_rest(states[MACROS - 2], MACROS - 2)
        tail_rest(states[MACROS - 1], MACROS - 1)

        outs = sp.tile([2, 8], dt.float32, tag="outs")
        nc.scalar.mul(out=outs[:], in_=acc_sb[:], mul=1.0 / N)
        nc.sync.dma_start(out_d.rearrange("b c -> c b"), outs[:])

    nc.compile()
    return nc


# ----------------------------------------------------------------------------
# v1 fallback (exact 16x gather) -- original kernel, kept verbatim
# ----------------------------------------------------------------------------

def _conv_chunk_weights(wf):
    """Block-Toeplitz per chunk: rows f-major (f, delta), cols (o_local, ch)."""
    Ws = []
    for (start, opc) in CHUNKS_V1:
        span = opc + KW - 1
        W = np.zeros((3 * span, opc * 12), np.float32)
        for f in range(3):
            for d in range(span):
                for o in range(opc):
                    k = d - o
                    if 0 <= k < KW:
                        W[f * span + d, o * 12:(o + 1) * 12] = wf[k, f, :]
        Ws.append(W)
    return Ws


def _d1_weights(w1):
    RA = np.zeros((72, 192), np.float32)
    RB = np.zeros((60, 160), np.float32)
    for p in range(6):
        RA[12 * p:12 * p + 12, 32 * p:32 * p + 32] = w1
    for p in range(5):
        RB[12 * p:12 * p + 12, 32 * p:32 * p + 32] = w1
    return RA, RB


def _build_host_data_v1(tokens, lengths, embed_w, conv_w, conv_b, w1, b1, w2, b2,
                     wc, bc):
    wf, bconv = _fold_weights(embed_w, conv_w, conv_b)
    Ws = _conv_chunk_weights(wf)
    RA, RB = _d1_weights(w1)
    wcd = (wc[:, 1] - wc[:, 0]).astype(np.float32)
    bcd = float(bc[1] - bc[0])

    consts = {
        "w_c0": Ws[0], "w_c1": Ws[1], "w_c2": Ws[2], "w_c3": Ws[3],
        "r1a": RA, "r1b": RB,
        "w2r": w2.astype(np.float32),
        "maskc": np.stack([(np.arange(16) == (p % 16)) for p in range(128)]
                          ).astype(np.float32),                  # [128,16]
        "ident": np.eye(128, dtype=np.float32),
        "bpool": np.tile(np.tile(bconv, NPOOL_V1)[None, :], (128, 1)),  # [128,132]
        "b1rep": np.tile(b1.astype(np.float32)[None, :], (128, 1)),
        "b2rep": np.tile(b2.astype(np.float32)[None, :], (128, 1)),
        "wcdrep": np.tile(wcd[None, :], (128, 1)),
        "ones1": np.ones((128, 1), np.float32),
        "biasv": np.tile(np.array([-2.0, -3.0, bcd, -bcd, 0.0, 0.0, 0.0, 0.0],
                                  np.float32)[None, :], (128, 1)),
    }

    tokens_r = tokens.reshape(N_CORES, SEQ_PER_CORE, L)
    lengths_r = lengths.reshape(N_CORES, SEQ_PER_CORE)
    per_core = []
    for c in range(N_CORES):
        lo, fr = _resize_tables(lengths_r[c])
        m = dict(consts)
        m["tok"] = np.ascontiguousarray(tokens_r[c])
        m["idx"] = lo.astype(np.uint16).reshape(TILES_V1, 128, TARGET)
        m["frac"] = fr.reshape(TILES_V1, 128, TARGET)
        per_core.append(m)
    return per_core, bcd


# ----------------------------------------------------------------------------
# device program
# ----------------------------------------------------------------------------

def _build_program_v1(bcd, repeat=1, ablate=()):
    import concourse.bass as bass
    import concourse.tile as tile
    from concourse import bacc, mybir

    dt = mybir.dt
    Alu = mybir.AluOpType
    Act = mybir.ActivationFunctionType
    Ax = mybir.AxisListType

    nc = bacc.Bacc("TRN2", target_bir_lowering=False, debug=False)

    def din(name, shape, dtype):
        return nc.dram_tensor(name, shape, dtype, kind="ExternalInput").ap()

    tok_d = din("tok", [SEQ_PER_CORE, L], dt.int32)
    idx_d = din("idx", [TILES_V1, 128, TARGET], dt.uint16)
    frac_d = din("frac", [TILES_V1, 128, TARGET], dt.float32)
    wc_d = [din(f"w_c{i}", [3 * (opc + 7), opc * 12], dt.float32)
            for i, (_, opc) in enumerate(CHUNKS_V1)]
    r1a_d = din("r1a", [72, 192], dt.float32)
    r1b_d = din("r1b", [60, 160], dt.float32)
    w2r_d = din("w2r", [32, 64], dt.float32)
    maskc_d = din("maskc", [128, 16], dt.float32)
    ident_d = din("ident", [128, 128], dt.float32)
    bpool_d = din("bpool", [128, 132], dt.float32)
    b1rep_d = din("b1rep", [128, 32], dt.float32)
    b2rep_d = din("b2rep", [128, 64], dt.float32)
    wcdrep_d = din("wcdrep", [128, 64], dt.float32)
    ones1_d = din("ones1", [128, 1], dt.float32)
    biasv_d = din("biasv", [128, 8], dt.float32)
    out_d = nc.dram_tensor("out", [B // N_CORES, 2], dt.float32,
                           kind="ExternalOutput").ap()

    with tile.TileContext(nc) as tc, ExitStack() as ctx:
        cpool = ctx.enter_context(tc.tile_pool(name="consts", bufs=1))
        iopool = ctx.enter_context(tc.tile_pool(name="io", bufs=2))
        gpool = ctx.enter_context(tc.tile_pool(name="gather", bufs=2))
        wpool = ctx.enter_context(tc.tile_pool(name="work", bufs=2))
        pspool = ctx.enter_context(tc.tile_pool(name="ps", bufs=2, space="PSUM"))
        accpool = ctx.enter_context(tc.tile_pool(name="acc", bufs=1, space="PSUM"))

        def cload(ap_d, shape, dtype, tag):
            t = cpool.tile(shape, dtype, tag=tag)
            nc.sync.dma_start(t[:], ap_d[:])
            return t

        Wc = [cload(wc_d[i], [3 * (opc + 7), opc * 12], dt.float32, f"cW{i}")
              for i, (_, opc) in enumerate(CHUNKS_V1)]
        R1A = cload(r1a_d, [72, 192], dt.float32, "cR1A")
        R1B = cload(r1b_d, [60, 160], dt.float32, "cR1B")
        W2R = cload(w2r_d, [32, 64], dt.float32, "cW2R")
        MASKC = cload(maskc_d, [128, 16], dt.float32, "cMASK")
        IDENT = cload(ident_d, [128, 128], dt.float32, "cID")
        BPOOL = cload(bpool_d, [128, 132], dt.float32, "cBP")
        B1REP = cload(b1rep_d, [128, 32], dt.float32, "cB1")
        B2REP = cload(b2rep_d, [128, 64], dt.float32, "cB2")
        WCDREP = cload(wcdrep_d, [128, 64], dt.float32, "cWCD")
        ONES1 = cload(ones1_d, [128, 1], dt.float32, "cON")
        BIASV = cload(biasv_d, [128, 8], dt.float32, "cBV")

        mean_ps = accpool.tile([2, TILES_V1], dt.float32)

        rep_ctx = tc.For_i(0, repeat, 1) if repeat > 1 else None
        if rep_ctx is not None:
            rep_ctx.__enter__()
        for t in range(TILES_V1):
            # ---- load tile inputs (tokens cast int32->fp32 via SWDGE) ----
            data3 = iopool.tile([128, L // 2, 2], dt.float32, tag="data3")
            nc.gpsimd.dma_start(
                data3[:],
                tok_d[t * 128:(t + 1) * 128, :].rearrange("p (n d) -> p n d", d=2))
            idxt = iopool.tile([128, TARGET], dt.uint16, tag="idxt")
            nc.sync.dma_start(idxt[:], idx_d[t])
            frct = iopool.tile([128, TARGET], dt.float32, tag="frct")
            nc.sync.dma_start(frct[:], frac_d[t])

            # ---- gather pairs: junk[p, i*16+k, e] = tok[p, lo[seq16k, i]+e] ----
            junk = gpool.tile([128, TARGET * 16, 2], dt.float32, tag="junk")
            if "gather" in ablate:
                nc.vector.memset(junk[:].rearrange("p a b -> p (a b)"), 1.0)
            else:
                for (i0, ni) in ((0, 32), (32, 32), (64, 32), (96, 4)):
                    nc.gpsimd.indirect_copy(
                        junk[:, i0 * 16:(i0 + ni) * 16, :],
                        data3[:],
                        idxt[:, i0:i0 + ni],
                        i_know_ap_gather_is_preferred=True)

            # ---- extract tlo/thi via masked grouped reduce ----
            jv = junk[:].rearrange("p (i k) e -> p i k e", k=16)
            mb = MASKC[:].rearrange("p (a k) -> p a k", a=1) \
                         .to_broadcast([128, TARGET, 16])
            prod = wpool.tile([128, TARGET, 16], dt.float32, tag="prod")
            lo3 = wpool.tile([128, 3, TARGET], dt.float32, tag="lo3")
            hi3 = wpool.tile([128, 3, TARGET], dt.float32, tag="hi3")
            nc.vector.tensor_tensor(out=prod[:], in0=jv[:, :, :, 0], in1=mb,
                                    op=Alu.mult)
            nc.vector.tensor_reduce(out=lo3[:, 0, :], in_=prod[:], axis=Ax.X,
                                    op=Alu.add)
            nc.vector.tensor_tensor(out=prod[:], in0=jv[:, :, :, 1], in1=mb,
                                    op=Alu.mult)
            nc.vector.tensor_reduce(out=hi3[:, 0, :], in_=prod[:], axis=Ax.X,
                                    op=Alu.add)

            # ---- features: f1 = t, f2 = relu(t-2), f3 = relu(t-3) ----
            for buf in (lo3, hi3):
                nc.scalar.activation(out=buf[:, 1, :], in_=buf[:, 0, :],
                                     func=Act.Relu, bias=BIASV[:, 0:1])
                nc.scalar.activation(out=buf[:, 2, :], in_=buf[:, 0, :],
                                     func=Act.Relu, bias=BIASV[:, 1:2])

            # ---- bilinear blend: d3 = frac*(hi3 - lo3); x3c per chunk ----
            d3 = wpool.tile([128, 3, TARGET], dt.float32, tag="d3")
            frb = frct[:].rearrange("p (a i) -> p a i", a=1) \
                         .to_broadcast([128, 3, TARGET])
            nc.vector.tensor_tensor(out=d3[:], in0=hi3[:], in1=lo3[:],
                                    op=Alu.subtract)
            nc.vector.tensor_tensor(out=d3[:], in0=d3[:], in1=frb, op=Alu.mult)

            # ---- conv: chunk-contiguous blend -> transpose -> matmul -> pool ----
            pooled = wpool.tile([128, 132], dt.float32, tag="pooled")
            for ci, (start, opc) in enumerate(CHUNKS_V1):
                span = opc + 7
                x3c = wpool.tile([128, 3, span], dt.float32, tag="x3c")
                nc.vector.tensor_tensor(out=x3c[:],
                                        in0=lo3[:, :, start:start + span],
                                        in1=d3[:, :, start:start + span],
                                        op=Alu.add)
                tp = pspool.tile([3 * 31, 128], dt.float32, tag="tp")
                nc.tensor.transpose(
                    out=tp[:3 * span, :],
                    in_=x3c[:].rearrange("p f s -> p (f s)"),
                    identity=IDENT[:])
                xtc = wpool.tile([3 * 31, 128], dt.float32, tag="xtc")
                nc.scalar.copy(out=xtc[:3 * span, :], in_=tp[:3 * span, :])
                y_ps = pspool.tile([128, opc * 12], dt.float32, tag="mm")
                nc.tensor.matmul(out=y_ps[:], lhsT=xtc[:3 * span, :],
                                 rhs=Wc[ci][:], start=True, stop=True)
                g = opc // 8
                yv = y_ps[:].rearrange("p (g o c) -> p g c o", g=g, o=8)
                nc.vector.tensor_reduce(
                    out=pooled[:, 36 * ci:36 * ci + 12 * g]
                        .rearrange("p (g c) -> p g c", g=g),
                    in_=yv, axis=Ax.X, op=Alu.max)

            # ---- bias + relu ----
            h = wpool.tile([128, 132], dt.float32, tag="h")
            nc.vector.tensor_tensor(out=h[:], in0=pooled[:], in1=BPOOL[:],
                                    op=Alu.add)
            nc.scalar.activation(out=h[:], in_=h[:], func=Act.Relu,
                                 bias=BIASV[:, 4:5])

            # ---- dense1 (block-diag) + global max over 11 pools ----
            htA_ps = pspool.tile([72, 128], dt.float32, tag="tp")
            nc.tensor.transpose(out=htA_ps[:], in_=h[:, 0:72], identity=IDENT[:])
            htA = wpool.tile([72, 128], dt.float32, tag="htA")
            nc.scalar.copy(out=htA[:], in_=htA_ps[:])
            htB_ps = pspool.tile([60, 128], dt.float32, tag="tp")
            nc.tensor.transpose(out=htB_ps[:], in_=h[:, 72:132], identity=IDENT[:])
            htB = wpool.tile([60, 128], dt.float32, tag="htB")
            nc.scalar.copy(out=htB[:], in_=htB_ps[:])

            h1a_ps = pspool.tile([128, 192], dt.float32, tag="mm")
            nc.tensor.matmul(out=h1a_ps[:], lhsT=htA[:], rhs=R1A[:],
                             start=True, stop=True)
            h1b_ps = pspool.tile([128, 160], dt.float32, tag="mm2")
            nc.tensor.matmul(out=h1b_ps[:], lhsT=htB[:], rhs=R1B[:],
                             start=True, stop=True)

            ga = wpool.tile([128, 32], dt.float32, tag="ga")
            gb = wpool.tile([128, 32], dt.float32, tag="gb")
            nc.vector.tensor_reduce(
                out=ga[:], in_=h1a_ps[:].rearrange("p (g o) -> p o g", g=6),
                axis=Ax.X, op=Alu.max)
            nc.vector.tensor_reduce(
                out=gb[:], in_=h1b_ps[:].rearrange("p (g o) -> p o g", g=5),
                axis=Ax.X, op=Alu.max)
            nc.vector.tensor_tensor(out=ga[:], in0=ga[:], in1=gb[:], op=Alu.max)
            nc.vector.tensor_tensor(out=ga[:], in0=ga[:], in1=B1REP[:], op=Alu.add)
            nc.scalar.activation(out=ga[:], in_=ga[:], func=Act.Relu,
                                 bias=BIASV[:, 4:5])

            # ---- dense2 + relu ----
            gt_ps = pspool.tile([32, 128], dt.float32, tag="tp")
            nc.tensor.transpose(out=gt_ps[:], in_=ga[:], identity=IDENT[:])
            gt = wpool.tile([32, 128], dt.float32, tag="gt")
            nc.scalar.copy(out=gt[:], in_=gt_ps[:])
            r2_ps = pspool.tile([128, 64], dt.float32, tag="mm2")
            nc.tensor.matmul(out=r2_ps[:], lhsT=gt[:], rhs=W2R[:],
                             start=True, stop=True)
            r2 = wpool.tile([128, 64], dt.float32, tag="r2")
            nc.vector.tensor_tensor(out=r2[:], in0=r2_ps[:], in1=B2REP[:],
                                    op=Alu.add)
            nc.scalar.activation(out=r2[:], in_=r2[:], func=Act.Relu,
                                 bias=BIASV[:, 4:5])

            # ---- classifier: zd = r2@wcd + bcd; probs = [sig(-zd), sig(zd)] ----
            pz = wpool.tile([128, 64], dt.float32, tag="pz")
            nc.vector.tensor_tensor(out=pz[:], in0=r2[:], in1=WCDREP[:],
                                    op=Alu.mult)
            zd = wpool.tile([128, 1], dt.float32, tag="zd")
            nc.vector.tensor_reduce(out=zd[:], in_=pz[:], axis=Ax.X, op=Alu.add)
            probs = wpool.tile([128, 2], dt.float32, tag="probs")
            nc.scalar.activation(out=probs[:, 1:2], in_=zd[:], func=Act.Sigmoid,
                                 bias=BIASV[:, 2:3])
            nc.scalar.activation(out=probs[:, 0:1], in_=zd[:], func=Act.Sigmoid,
                                 bias=BIASV[:, 3:4], scale=-1.0)

            # ---- per-tile node-sum: mean_ps[:, t] = probs.T @ ones ----
            nc.tensor.matmul(out=mean_ps[:, t:t + 1], lhsT=probs[:],
                             rhs=ONES1[:], start=True, stop=True)

        if rep_ctx is not None:
            rep_ctx.__exit__(None, None, None)

        # ---- finalize: combine tile pairs, /256, write [8, 2] ----
        sums = wpool.tile([2, B // N_CORES], dt.float32, tag="sums")
        nc.vector.tensor_reduce(
            out=sums[:], in_=mean_ps[:].rearrange("p (b t) -> p b t", t=2),
            axis=Ax.X, op=Alu.add)
        outs = wpool.tile([2, B // N_CORES], dt.float32, tag="outs")
        nc.scalar.mul(out=outs[:], in_=sums[:], mul=1.0 / N)
        nc.sync.dma_start(out_d.rearrange("b c -> c b"), outs[:])

    nc.compile()
    return nc




# ----------------------------------------------------------------------------
# entry point
# ----------------------------------------------------------------------------

def _run(nc, per_core):
    from concourse.bass_utils import run_bass_kernel_spmd
    global LAST_EXEC_NS, LAST_TRACE_PATH
    if TRACE:
        res = run_bass_kernel_spmd(nc, per_core, list(range(N_CORES)),
                                   trace=True, tmpdir=TRACE_DIR)
        LAST_EXEC_NS = res.exec_time_ns
        if res.instructions_and_trace is not None:
            LAST_TRACE_PATH = res.instructions_and_trace[1]
    else:
        res = run_bass_kernel_spmd(nc, per_core, list(range(N_CORES)))
    return np.concatenate([res.results[c]["out"] for c in range(N_CORES)],
                          axis=0).astype(np.float32)


def kernel(**inputs):
    tokens = np.asarray(inputs["tokens"])
    lengths = np.asarray(inputs["lengths"])
    args = [np.asarray(inputs[k]) for k in
            ("embed_w", "conv_w", "conv_b", "w1", "b1", "w2", "b2", "wc", "bc")]

    consts, bcd = build_consts(*args)
    tokens_r = tokens.reshape(N_CORES, SEQ_PER_CORE, L)
    lengths_r = lengths.reshape(N_CORES, SEQ_PER_CORE)
    per_core = []
    ok = True
    for c in range(N_CORES):
        md, mo = plan_core(tokens_r[c], lengths_r[c])
        if mo > W - 1:
            ok = False
            break
        d = dict(consts)
        d.update(md)
        per_core.append(d)

    if ok:
        key = ("v2", round(bcd, 8))
        if key not in _CACHE:
            _CACHE[key] = build_program(bcd)
        return _run(_CACHE[key], per_core)

    # fallback: exact-gather v1 program
    per_core_v1, bcd = _build_host_data_v1(tokens, lengths, *args)
    key = ("v1", round(bcd, 8))
    if key not in _CACHE:
        _CACHE[key] = _build_program_v1(bcd)
    return _run(_CACHE[key], per_core_v1)
